# revision 14
# baseline (speedup 1.0000x reference)
"""Trainium2 Bass kernel for nn_LoRAExpert (moe_routing).

Per token t (expert e_t from contiguous group_sizes, adapter a_t):

    out[t] = x[t] @ W[e_t] + s_{a_t} * (x[t] @ A[a_t, e_t]) @ B[a_t, e_t]

Strategy (expert-parallel over 8 NeuronCores):
  - Host routes tokens: x is already expert-sorted, so core e gets the
    contiguous slice x[off_e : off_e + gs_e], padded to a common `cap`.
  - LoRA routing trick: with A=8 adapters and rank R=16, the per-expert
    concatenation A_cat = [A[0,e] .. A[7,e]] is [1024, 128]. Compute
    inter_all = x @ A_cat densely for ALL adapters, then multiply by a
    per-token mask M[j, t] = s_{a_t} * (j in adapter-a_t block) and feed
    the masked inter into B_cat = [B[0,e]; ..; B[7,e]] ([128, 1024]).
    This turns the ragged adapter grouping into two dense matmuls and
    one elementwise mask — no on-device sorting or control flow.
  - The B-side matmul accumulates into the same PSUM tile as the base
    matmul, so base + lora is free.
  - All matmul operands are cast to bf16 on the host (fp32 PSUM
    accumulation on the PE); the output is stored bf16 and widened to
    fp32 on the host (absmax error stays ~3e-3, well under the 2e-2
    gate, and HBM write traffic halves).

Performance structure (v2): input DMAs are consolidated into ~10 large
transfers ordered so the first-needed bytes land first (x tokens 0-511
k-lo, W k-lo, A_cat, x k-hi, W k-hi, B_cat, mask, remaining x), the
first three token tiles are emitted as k-split jobs so the PE can start
on the k0-3 half of W while k4-7 is still in flight, and the lora/copy
"finish" stage of tile j is emitted after the base stage of tile j+2 so
the PE never has to wait on the mask/intermediate path. A ~4us burst of
throwaway matmuls at the head keeps the PE HAM clock gate warm through
the DMA lead-in.

The kernel is compiled for cap = max(group_sizes) rounded up to 128 and
cached per cap. All 8 cores run one SPMD program; per-core data differs
only through the input maps.
"""

import numpy as np

T, E, IN, OUT, A, R = 16384, 8, 1024, 1024, 8, 16
NCORES = 8
AR = A * R  # 128
KC = IN // 128  # 8 contraction chunks
OC = OUT // 512  # 2 output column chunks
NWARM = 16  # HAM warmup matmuls (N=512 each) during the DMA lead-in

_compiled_cache: dict[int, object] = {}


# ---------------------------------------------------------------------------
# walrus in this container accepts at most 1 sync-wait command per
# instruction; Tile attaches more. Split excess waits onto no-ops.
# ---------------------------------------------------------------------------


def _apply_tile_wait_patch():
    import bass_rust
    import concourse.tile as tile
    from concourse import mybir
    from concourse.vector_clock import ScopedClock

    if getattr(tile.TileContext, "_wait_split_patched", False):
        return

    MAX_WAITS = 1

    def _split_excess_waits(nc):
        for fn in nc.m.functions:
            for blk in fn.blocks:
                insts = blk.instructions  # live list
                i = 0
                while i < len(insts):
                    inst = insts[i]
                    si = inst.sync_info
                    if si is not None and len(si.on_wait) > MAX_WAITS:
                        waits = list(si.on_wait)
                        keep = waits[-MAX_WAITS:]
                        excess = waits[:-MAX_WAITS]
                        inst.sync_info = bass_rust.SyncInfo(
                            on_wait=keep, on_update=list(si.on_update)
                        )
                        pos = i
                        for k in range(0, len(excess), MAX_WAITS):
                            nop = mybir.InstNoOp(
                                name=f"{inst.name}-hoistw{k}",
                                engine=inst.engine,
                                bass_nofuse=True,
                                sync_info=mybir.SyncInfo(
                                    on_wait=excess[k : k + MAX_WAITS], on_update=[]
                                ),
                            )
                            insts.insert(pos, nop)
                            pos += 1
                            i += 1
                    i += 1

    def _split_drain_and_barrier(self, tick_clock, wait_clock):
        nc = self.nc
        drain_inst = nc.sync.drain()
        wait_clock.add_sem_waits(
            drain_inst.ins, ScopedClock({None: tick_clock.global_clock})
        )
        si = drain_inst.ins.sync_info
        if si is not None and len(si.on_wait) > MAX_WAITS:
            waits = list(si.on_wait)
            drain_inst.ins.sync_info = bass_rust.SyncInfo(
                on_wait=waits[:MAX_WAITS], on_update=list(si.on_update)
            )
            for k in range(MAX_WAITS, len(waits), MAX_WAITS):
                extra = nc.sync.drain()
                extra.ins.sync_info = bass_rust.SyncInfo(
                    on_wait=waits[k : k + MAX_WAITS], on_update=[]
                )

        import os as _os

        nc.all_engine_barrier()
        assert self.sems is not None
        popped = nc._tile_sem_poison_stack.pop()
        assert popped is self._sem_poison
        nc.clear_and_free_semaphores(list(self.sems.allocated().values()))
        if _os.environ.get("LORA_LEAN_TAIL", "1") != "1":
            # Second barrier only matters for kernels that continue past
            # the TileContext; ours ends here (sem clears trail on gpsimd).
            nc.all_engine_barrier()

        _split_excess_waits(nc)

    tile.TileContext._drain_and_barrier = _split_drain_and_barrier
    tile.TileContext._wait_split_patched = True


# ---------------------------------------------------------------------------
# NEFF postamble slimming: NRT injects a per-semaphore "=0" epilogue for
# every semaphore the NEFF does not declare runtime-reserved (~250
# EVENT_SEMAPHORE instructions, ~6us serialized across the engines).
# This kernel already returns every semaphore it touches to zero (Tile's
# EVENT_SEMAPHORE_RANGE_CLEAR + the self-zeroing barrier protocol), so
# the injected epilogue is pure dead time. Declaring all 256 semaphores
# runtime-reserved in def.json makes NRT skip it.
# ---------------------------------------------------------------------------


def _apply_neff_sem_patch():
    import os

    if os.environ.get("LORA_SEM_PATCH", "1") != "1":
        return
    import io
    import json
    import tarfile
    import tempfile

    import concourse.bass2jax as b2j
    from concourse import neff as cneff

    if getattr(b2j, "_lora_sem_patched", False):
        return

    orig = b2j.rename_neff_tensors_and_patch_header

    def patched(neff_path: str, mapping: dict) -> bytes:
        data = orig(neff_path, mapping)
        old_header, tar_data = data[:1024], data[1024:]
        with tempfile.TemporaryDirectory() as repack_dir:
            with tarfile.open(fileobj=io.BytesIO(tar_data), mode="r") as t:
                t.extractall(repack_dir)
            def_path = f"{repack_dir}/sg00/def.json"
            with open(def_path) as f:
                dj = json.load(f)
            dj["runtime_semaphore_count"] = 256
            with open(def_path, "w") as f:
                json.dump(dj, f)
            buf = io.BytesIO()
            with tarfile.open(fileobj=buf, mode="w") as t:
                t.add(repack_dir, arcname=".", filter=b2j._reset_tarinfo)
            new_tar = buf.getvalue()
            new_header = cneff.make_deterministic_neff_header(
                old_neff_header=old_header, new_neff_data=new_tar
            )
        return new_header + new_tar

    b2j.rename_neff_tensors_and_patch_header = patched
    b2j._lora_sem_patched = True


# ---------------------------------------------------------------------------
# Bass program (one SPMD NeuronCore program, parameterized by cap)
# ---------------------------------------------------------------------------


def _token_chunks(cap: int):
    """DMA token ranges for x beyond the first 512: 512-wide, last merged."""
    chunks = []
    s = 512
    while s < cap:
        e = min(s + 512, cap)
        chunks.append((s, e))
        s = e
    if len(chunks) >= 2 and chunks[-1][1] - chunks[-1][0] < 256:
        last = chunks.pop()
        chunks[-1] = (chunks[-1][0], last[1])
    return chunks


def _build(cap: int):
    import concourse.bass as bass
    import concourse.tile as tile
    from concourse import mybir

    _apply_tile_wait_patch()

    ntt = cap // 128  # token tiles
    ngr = (cap + 511) // 512  # phase-1 groups of up to 512 tokens
    assert ntt >= 4 and cap >= 512

    bf16 = mybir.dt.bfloat16
    f32 = mybir.dt.float32

    nc = bass.Bass()
    XT = nc.dram_tensor("xt", [128, KC, cap], bf16, kind="ExternalInput")
    W = nc.dram_tensor("w", [128, KC, OUT], bf16, kind="ExternalInput")
    ACAT = nc.dram_tensor("acat", [128, KC, AR], bf16, kind="ExternalInput")
    BCAT = nc.dram_tensor("bcat", [AR, OUT], bf16, kind="ExternalInput")
    MASKT = nc.dram_tensor("maskt", [AR, cap], bf16, kind="ExternalInput")
    OUTD = nc.dram_tensor("out", [cap, OUT], bf16, kind="ExternalOutput")

    KH = KC // 2  # k-chunk half

    def gslice(g):
        t0 = g * 512
        return t0, min(512, cap - t0)

    with tile.TileContext(nc) as tc:
        with (
            tc.tile_pool(name="big", bufs=1) as big,
            tc.tile_pool(name="outp", bufs=4) as outp,
            tc.tile_pool(name="psi", bufs=2, space="PSUM") as psi,
            tc.tile_pool(name="pso", bufs=6, space="PSUM") as pso,
        ):
            # SBUF tiles
            xt_sb = big.tile([128, KC, cap], bf16)
            w_sb = big.tile([128, KC, OUT], bf16)
            a_sb = big.tile([128, KC, AR], bf16)
            b_sb = big.tile([AR, OUT], bf16)
            maskt_sb = big.tile([AR, cap], bf16)
            interm_sb = big.tile([AR, cap], bf16)
            warm_sb = big.tile([128, 512], bf16)

            # Input DMAs for the critical lead-in set, one per large
            # contiguous-ish region, in the order the compute consumes
            # them. Issue occupies the Sync queue ~0.65us per
            # instruction, so fewer/larger is better.
            from concourse.tile_rust import add_dep_helper

            def after(d, gate):
                add_dep_helper(
                    d.ins, gate.ins, sync=True, reason="lead-in DMA ordering"
                )
                return d

            # Lead-in DMAs are chained so the HBM bandwidth serves the
            # first-needed bytes first: x(k-lo)+W(k-lo) immediately (they
            # gate the first token tiles), everything else only once W-lo
            # has fully landed, and the x tail only after W-hi.
            nc.sync.dma_start(xt_sb[:, 0:KH, 0:512], XT[:, 0:KH, 0:512])
            wlo = nc.sync.dma_start(w_sb[:, 0:KH, :], W[:, 0:KH, :])
            after(nc.sync.dma_start(a_sb[:], ACAT[:]), wlo)
            after(
                nc.sync.dma_start(xt_sb[:, KH:KC, 0:512], XT[:, KH:KC, 0:512]),
                wlo,
            )
            whi = after(nc.sync.dma_start(w_sb[:, KH:KC, :], W[:, KH:KC, :]), wlo)
            after(nc.sync.dma_start(b_sb[:], BCAT[:]), whi)
            after(nc.sync.dma_start(maskt_sb[:], MASKT[:]), whi)
            # Remaining x chunks go through the otherwise-idle GpSimd
            # SWDGE queue (their 128x8-run descriptor patterns are
            # expensive to generate on the Sync HWDGE queue).
            for t0, t1 in _token_chunks(cap):
                after(
                    nc.gpsimd.dma_start(xt_sb[:, :, t0:t1], XT[:, :, t0:t1]),
                    whi,
                )

            # HAM warmup: keep the PE busy through the DMA lead-in so the
            # clock gate is at 8/8 when real matmuls start. memset on the
            # otherwise-idle GpSimd engine so the PE isn't gated on the
            # busier Vector queue.
            nc.gpsimd.memset(warm_sb[:], 0.0)
            wps = psi.tile([128, 512], f32, name="warm", tag="psi")
            for i in range(NWARM):
                nc.tensor.matmul(
                    wps[:], warm_sb[:, 0:128], warm_sb[:],
                    start=(i == 0), stop=(i == NWARM - 1),
                )

            pss: dict[int, list] = {}
            p1_tiles: dict[int, object] = {}
            p1_done: set[int] = set()

            def alloc(j):
                pss[j] = [
                    pso.tile([128, 512], f32, name=f"pso{j}_{oc}", tag="pso")
                    for oc in range(OC)
                ]

            def base(j, k0, k1):
                ts0 = j * 128
                for k in range(k0, k1):
                    for oc in range(OC):
                        nc.tensor.matmul(
                            pss[j][oc][:],
                            xt_sb[:, k, ts0 : ts0 + 128],
                            w_sb[:, k, oc * 512 : oc * 512 + 512],
                            start=(k == 0),
                            stop=False,
                        )

            def phase1_mm(g, k0, k1):
                t0, wg = gslice(g)
                if g not in p1_tiles:
                    p1_tiles[g] = psi.tile(
                        [128, 512], f32, name=f"psi{g}", tag="psi"
                    )
                ps = p1_tiles[g]
                for k in range(k0, k1):
                    nc.tensor.matmul(
                        ps[:, :wg],
                        a_sb[:, k, :],
                        xt_sb[:, k, t0 : t0 + wg],
                        start=(k == 0),
                        stop=(k == KC - 1),
                    )

            def phase1_mask(g):
                t0, wg = gslice(g)
                nc.vector.scalar_tensor_tensor(
                    interm_sb[:, t0 : t0 + wg],
                    p1_tiles[g][:, :wg],
                    1.0,
                    maskt_sb[:, t0 : t0 + wg],
                    mybir.AluOpType.mult,
                    mybir.AluOpType.mult,
                )
                p1_done.add(g)

            def ensure_phase1(g):
                if g not in p1_done and g < ngr:
                    phase1_mm(g, 0, KC)
                    phase1_mask(g)

            def fin(j):
                ts0 = j * 128
                for oc in range(OC):
                    nc.tensor.matmul(
                        pss[j][oc][:],
                        interm_sb[:, ts0 : ts0 + 128],
                        b_sb[:, oc * 512 : oc * 512 + 512],
                        start=False,
                        stop=True,
                    )
                o_sb = outp.tile([128, OUT], bf16, name=f"o{j}", tag="outp")
                nc.scalar.copy(o_sb[:, 0:512], pss[j][0][:])
                nc.vector.tensor_copy(o_sb[:, 512:OUT], pss[j][1][:])
                if j >= ntt - 2:
                    # Tail latency: ship each half as soon as its copy
                    # lands, from two different HWDGE queues, instead of
                    # one combined transfer after both copies.
                    nc.scalar.dma_start(
                        OUTD[ts0 : ts0 + 128, 0:512], o_sb[:, 0:512]
                    )
                    nc.sync.dma_start(
                        OUTD[ts0 : ts0 + 128, 512:OUT], o_sb[:, 512:OUT]
                    )
                else:
                    nc.sync.dma_start(OUTD[ts0 : ts0 + 128, :], o_sb[:])

            # Prologue: k-split jobs for tiles 0-2 so the PE can run on
            # the lo half of W while the hi half is still in flight.
            alloc(0)
            base(0, 0, KH)
            alloc(1)
            base(1, 0, KH)
            phase1_mm(0, 0, KH)
            alloc(2)
            base(2, 0, KH)
            phase1_mm(0, KH, KC)
            phase1_mask(0)
            base(0, KH, KC)
            base(1, KH, KC)
            base(2, KH, KC)
            fin(0)

            # Steady state: finish stage trails the base stage by 2 tiles.
            for j in range(3, ntt):
                alloc(j)
                base(j, 0, KC)
                g = j // 4
                if g >= 1:
                    ensure_phase1(g)
                if j == ntt - 1:
                    for g2 in range(1, ngr):
                        ensure_phase1(g2)
                fin(j - 2)
            fin(ntt - 2)
            fin(ntt - 1)

    return nc


def _get_compiled(cap: int):
    if cap not in _compiled_cache:
        _compiled_cache[cap] = _build(cap)
    return _compiled_cache[cap]


# ---------------------------------------------------------------------------
# Host-side routing + execution
# ---------------------------------------------------------------------------


def _reference_numpy(x, group_sizes, adapter_indices_sorted, weight, lora_A, lora_B, lora_scaling):
    """Fallback replicating the jax reference exactly (only used for
    degenerate group_sizes that do not sum to T)."""
    x = np.asarray(x, np.float32)
    gs = np.asarray(group_sizes, np.int64)
    adapter = np.asarray(adapter_indices_sorted, np.int64)
    out = np.zeros((x.shape[0], weight.shape[2]), np.float32)
    # base: ragged_dot semantics (groups from cumsum, tail rows -> 0)
    offs = np.minimum(np.concatenate([[0], np.cumsum(gs)]), x.shape[0])
    for e in range(E):
        s, t = offs[e], offs[e + 1]
        if t > s:
            out[s:t] = x[s:t] @ weight[e]
    # lora: expert ids via repeat padded with the final value
    rep = np.repeat(np.arange(E), np.maximum(gs, 0))[: x.shape[0]]
    if rep.size == 0:
        rep = np.zeros(x.shape[0], np.int64)
    elif rep.size < x.shape[0]:
        rep = np.concatenate(
            [rep, np.full(x.shape[0] - rep.size, rep[-1], np.int64)]
        )
    for t in range(x.shape[0]):
        e, a = rep[t], adapter[t]
        inter = x[t] @ lora_A[a, e]
        out[t] += lora_scaling[a] * (inter @ lora_B[a, e])
    return out


def kernel(x, group_sizes, adapter_indices_sorted, weight, lora_A, lora_B, lora_scaling):
    import ml_dtypes

    x = np.ascontiguousarray(np.asarray(x, np.float32))
    weight = np.asarray(weight, np.float32)
    lora_A = np.asarray(lora_A, np.float32)
    lora_B = np.asarray(lora_B, np.float32)
    scaling = np.asarray(lora_scaling, np.float32)
    gs = np.asarray(group_sizes).astype(np.int64)
    adapter = np.asarray(adapter_indices_sorted).astype(np.int64)

    if gs.sum() != T or (gs < 0).any() or int(gs.max()) <= 384:
        return _reference_numpy(
            x, gs, adapter, weight, lora_A, lora_B, scaling
        )

    from concourse.bass_utils import run_bass_kernel_spmd

    _apply_neff_sem_patch()

    bf = ml_dtypes.bfloat16
    cap = int(max(512, -(-int(gs.max()) // 128) * 128))
    nc = _get_compiled(cap)

    offs = np.concatenate([[0], np.cumsum(gs)])
    in_maps = []
    for e in range(NCORES):
        n = int(gs[e])
        s = int(offs[e])
        xe = np.zeros((cap, IN), np.float32)
        xe[:n] = x[s : s + n]
        # [128, KC, cap]: XT[p, k, t] = x_e[t, 128k+p]
        xt = np.ascontiguousarray(
            xe.T.reshape(KC, 128, cap).transpose(1, 0, 2).astype(bf)
        )
        w = np.ascontiguousarray(
            weight[e].reshape(KC, 128, OUT).transpose(1, 0, 2).astype(bf)
        )
        # A_cat[:, a*R+r] = lora_A[a, e, :, r] -> [128, KC, AR]
        acat_full = lora_A[:, e].transpose(1, 0, 2).reshape(IN, AR)
        acat = np.ascontiguousarray(
            acat_full.reshape(KC, 128, AR).transpose(1, 0, 2).astype(bf)
        )
        bcat = np.ascontiguousarray(lora_B[:, e].reshape(AR, OUT).astype(bf))
        ae = adapter[s : s + n]
        m = np.zeros((A, cap), np.float32)
        m[ae, np.arange(n)] = scaling[ae]
        maskt = np.ascontiguousarray(np.repeat(m, R, axis=0).astype(bf))
        in_maps.append(
            {"xt": xt, "w": w, "acat": acat, "bcat": bcat, "maskt": maskt}
        )

    res = run_bass_kernel_spmd(nc, in_maps, list(range(NCORES)))

    out = np.empty((T, OUT), np.float32)
    for e in range(NCORES):
        n = int(gs[e])
        if n:
            out[int(offs[e]) : int(offs[e]) + n] = (
                res.results[e]["out"][:n].astype(np.float32)
            )
    return out
